# revision 30
# baseline (speedup 1.0000x reference)
"""Trainium2 Bass kernel for nn_Backward_12094627905824 (MLP trunk + gumbel-argmax
mixture sampling). Data-parallel over 8 NeuronCores: batch B=262144 is sharded
32768 rows/core; the small MLP / head weights are replicated.

Math per batch row b (reference semantics):
  h = relu chain: 3 -> 128 -> 256 -> 200
  mu/sig/pai[g,d] = heads (25 comps x 4 dims), pai/sigma through abs
  idx[d] = argmax_g log(pai+1e-12) + gumbel[b,g,d]
  out[b,d] = rand[b,d]*|sig[idx,d]| + mu[idx,d]

On-device reformulation (argmax-invariant): score = |pai_raw| * exp(gumbel);
selection via one-hot (score >= rowmax); out = sum_g onehot * z where
z = mu + rand*|sig| is folded BEFORE the masked reduce (one select instead of
two).

v2 engine plan (per 512-row tile, 64 tiles/core):
  PE   : 13 matmuls, all biases folded in (x-pack carries no ones; trunk
         biases are added exactly in f32 by Pool/Act; head biases enter via
         two ones-rows in the h3a SBUF tile against bias hi/lo rows of the
         packed head weights - same precision as the baseline's hi/lo ones
         matmul, but zero extra PE cycles).
  Act  : h2a/h2b fused bias+relu (psum->sbuf), exp(gumbel).
  Pool : h1/h3a/h3b fused bias+relu via tensor_scalar(add bias, max 0),
         |pai|, |sig| via abs_max, z1 = |sig|*rand, z = z1 + mu.
  DVE  : score mul, rowmax reduce, one-hot is_ge, z*onehot, masked-sum.
  DMA  : everything batched: x/weights/biases/rand/out are ONE DMA each
         (host-side relayout packs them 128-partitions-tall), gumbel in 8
         contiguous group DMAs of 8 tiles each.
"""
import numpy as np

import concourse.bass as bass
import concourse.mybir as mybir
import bass_rust
from concourse.tile import TileContext
from concourse.bass_utils import run_bass_kernel_spmd

NCORES = 8
B, G, D = 262144, 25, 4
GD = G * D                       # 100
H1, H2, H3 = 128, 256, 200
BS = B // NCORES                 # 32768 rows per core
NB = 512                         # batch columns per compute tile
NT = BS // NB                    # 64 tiles
NSUB = NB // 128                 # 4 sub-blocks of 128 rows
CH = 8                           # tiles per gumbel DMA group
NGRP = NT // CH                  # 8 groups

F32 = mybir.dt.float32
F32R = mybir.dt.float32r
BF16 = mybir.dt.bfloat16

# weight-pack column map (f32r matmul operands, 128 partitions)
# W1 region: 8 shifted variants (K=32 trick): variant a has W1.T in rows
# 4a..4a+2 of a [32,128] block, zeros elsewhere; replicated at bases 0 and 64
# so lhsT.base matches the x rhs base (PE quadrant rule).
WP_W1 = 0          # [{0:32,64:96}, 0:1024]  8 x [32,128] shifted W1.T blocks
WP_W2 = 1024       # [0:128, +0:256]  W2.T (h2a cols 0:128, h2b cols 128:256)
WP_W3A = 1280      # [0:128, +0:200]  W3.T[h2 feats 0:128]  (h3a 0:100, h3b 100:200)
WP_W3B = 1480      # [0:128, +0:200]  W3.T[h2 feats 128:256]
WP_WHA = 1680      # [0:102, +0:300]  heads for h3 feats 0:100 + bias hi/lo rows
WP_WHB = 1980      # [0:100, +0:300]  heads for h3 feats 100:200
WP_ONES = 2280     # [0:1, +0:512]    ones row (rhs of the b3-preseed matmuls)
WP_B3 = 2792       # [0:1, +0:200]    b3 as a row (lhsT of the preseed matmuls)
WP_COLS = 2992


def _split_multi_waits(nc):
    # walrus CoreV3 codegen accepts only one sync-wait per instruction; Tile's
    # exit drain waits once per active proc. Split into single-wait drains.
    for bb in nc.main_func.blocks:
        insts = list(bb.instructions)
        out = []
        changed = False
        for inst in insts:
            si = inst.sync_info
            if si is not None and len(si.on_wait) > 1:
                waits = list(si.on_wait)
                for k, w in enumerate(waits[:-1]):
                    d = mybir.InstDrain(name=f"{inst.name}-sw{k}", ins=[], outs=[])
                    d.engine = inst.engine
                    d.sync_info = bass_rust.SyncInfo(on_wait=[w], on_update=[])
                    nc.register_instruction(d)
                    out.append(d)
                si.on_wait = [waits[-1]]
                changed = True
            out.append(inst)
        if changed:
            bb.instructions = out


def _build_nc():
    nc = bass.Bass(trn_type="TRN2")

    xp = nc.dram_tensor("xp", [96, 2048], F32R, kind="ExternalInput")
    ones2 = nc.dram_tensor("ones2", [2, 512], F32R, kind="ExternalInput")
    wp = nc.dram_tensor("wp", [128, WP_COLS], F32R, kind="ExternalInput")
    bp = nc.dram_tensor("bp", [128, 5], F32, kind="ExternalInput")
    gum = nc.dram_tensor("gum", [128, NT * NSUB * GD], F32, kind="ExternalInput")
    rnd = nc.dram_tensor("rnd", [128, NT * NSUB * D], F32, kind="ExternalInput")
    out_d = nc.dram_tensor("out", [128, NT * NSUB * D], F32, kind="ExternalOutput")

    AX = mybir.AxisListType.X
    OP = mybir.AluOpType
    RELU = mybir.ActivationFunctionType.Relu
    EXP = mybir.ActivationFunctionType.Exp

    from contextlib import ExitStack
    with TileContext(nc) as tc, ExitStack() as ctx:
        const = ctx.enter_context(tc.tile_pool(name="const", bufs=1))
        io = ctx.enter_context(tc.tile_pool(name="io", bufs=2))
        act = ctx.enter_context(tc.tile_pool(name="act", bufs=3))
        samp = ctx.enter_context(tc.tile_pool(name="samp", bufs=3))
        # trunk psum: 2 rotating 1-bank tiles hold h1p/h2ap/h2bp
        pT = ctx.enter_context(tc.tile_pool(name="pT", bufs=2, space="PSUM"))
        # h3 psum: one 2-bank [100,2,512] tile (b3 preseeded by K=1 matmuls)
        pH3 = ctx.enter_context(tc.tile_pool(name="pH3", bufs=1, space="PSUM"))
        # heads psum: one 4-bank tile, freed by the per-tile drain ops
        pHD = ctx.enter_context(tc.tile_pool(name="pHD", bufs=1, space="PSUM"))

        # --- load packed inputs (Act queue: x/weights/biases/rand; SP: gum) ---
        xp_s = const.tile([96, 2048], F32R, tag="xp")
        nc.scalar.dma_start(out=xp_s, in_=xp[:, :])
        wp_s = const.tile([128, WP_COLS], F32R, tag="wp")
        nc.scalar.dma_start(out=wp_s, in_=wp[:, :])
        bp_s = const.tile([128, 5], F32, tag="bp")
        nc.scalar.dma_start(out=bp_s, in_=bp[:, :])
        rnd_s = const.tile([128, NT * NSUB * D], F32, tag="rnd")
        nc.scalar.dma_start(out=rnd_s, in_=rnd[:, :])
        outacc = const.tile([128, NT * NSUB * D], F32, tag="outacc")

        # rotating-state dicts keyed by tile index
        live = {}

        def st_gum(g):
            gum_s = io.tile([128, CH, NSUB, GD], F32, tag="gum")
            nc.sync.dma_start(
                out=gum_s,
                in_=gum[:, g * CH * NSUB * GD:(g + 1) * CH * NSUB * GD]
                .rearrange("p (c s e) -> p c s e", c=CH, s=NSUB))
            live[("gum", g)] = gum_s

        def st_h1(i):
            c2 = i // 2
            xt, xq, xa = c2 // 16, (c2 % 16) // 8, c2 % 8
            xcol = 1024 * xt + (i % 2) * 512
            h1p = pT.tile([128, 512], F32, tag="pt")
            nc.tensor.matmul(h1p,
                             lhsT=wp_s[64 * xq:64 * xq + 32,
                                       WP_W1 + 128 * xa:WP_W1 + 128 * (xa + 1)],
                             rhs=xp_s[64 * xq:64 * xq + 32, xcol:xcol + 512],
                             start=True, stop=True)
            h1s = act.tile([128, 512], F32R, tag="h1")
            nc.scalar.activation(h1s, h1p, func=RELU, bias=bp_s[:, 0:1], scale=1.0)
            live[("h1", i)] = h1s

        def st_h2(i):
            h1s = live.pop(("h1", i))
            h2ap = pT.tile([128, 512], F32, tag="pt")
            nc.tensor.matmul(h2ap, lhsT=wp_s[0:128, WP_W2:WP_W2 + 128],
                             rhs=h1s, start=True, stop=True)
            h2bp = pT.tile([128, 512], F32, tag="pt")
            nc.tensor.matmul(h2bp, lhsT=wp_s[0:128, WP_W2 + 128:WP_W2 + 256],
                             rhs=h1s, start=True, stop=True)
            h2s = act.tile([128, 2, 512], F32R, tag="h2")
            nc.scalar.activation(h2s[:, 0, :], h2ap, func=RELU,
                                 bias=bp_s[:, 1:2], scale=1.0)
            nc.scalar.activation(h2s[:, 1, :], h2bp, func=RELU,
                                 bias=bp_s[:, 2:3], scale=1.0)
            live[("h2", i)] = h2s

        def st_h3(i):
            h2s = live.pop(("h2", i))
            h3p = pH3.tile([100, 2, 512], F32, tag="h3p")
            # b3 preseed: psum := b3 (outer product b3-row x ones-row), then
            # the accumulating W3 matmuls add on top -> bias costs no drain op
            nc.tensor.matmul(h3p[:, 0, :], lhsT=wp_s[0:1, WP_B3:WP_B3 + 100],
                             rhs=wp_s[0:1, WP_ONES:WP_ONES + 512],
                             start=True, stop=False)
            nc.tensor.matmul(h3p[:, 0, :], lhsT=wp_s[0:128, WP_W3A:WP_W3A + 100],
                             rhs=h2s[:, 0, :], start=False, stop=False)
            nc.tensor.matmul(h3p[:, 0, :], lhsT=wp_s[0:128, WP_W3B:WP_W3B + 100],
                             rhs=h2s[:, 1, :], start=False, stop=True)
            nc.tensor.matmul(h3p[:, 1, :], lhsT=wp_s[0:1, WP_B3 + 100:WP_B3 + 200],
                             rhs=wp_s[0:1, WP_ONES:WP_ONES + 512],
                             start=True, stop=False)
            nc.tensor.matmul(h3p[:, 1, :], lhsT=wp_s[0:128, WP_W3A + 100:WP_W3A + 200],
                             rhs=h2s[:, 0, :], start=False, stop=False)
            nc.tensor.matmul(h3p[:, 1, :], lhsT=wp_s[0:128, WP_W3B + 100:WP_W3B + 200],
                             rhs=h2s[:, 1, :], start=False, stop=True)
            h3s = act.tile([102, 2, 512], F32R, tag="h3s")
            if i < 3:
                # ones rows for the head-bias hi/lo trick; each of the 3
                # rotating buffers is initialized once and never clobbered
                # (the relu below only writes rows 0:100). DMA because
                # vector-engine writes must start at partition 0/32/64/96.
                nc.scalar.dma_start(out=h3s[100:102, 0, :], in_=ones2[:, :])
            # ONE bias-free relu drains both h3 chunks
            nc.scalar.activation(h3s[0:100, :, :], h3p, func=RELU, scale=1.0)
            live[("h3", i)] = h3s

        def st_heads(i):
            h3s = live.pop(("h3", i))
            hp = pHD.tile([128, NSUB, 512], F32, tag="hp")
            for s in range(NSUB):
                c0, c1 = s * 128, (s + 1) * 128
                nc.tensor.matmul(hp[:, s, 0:300], lhsT=h3s[0:102, 0, c0:c1],
                                 rhs=wp_s[0:102, WP_WHA:WP_WHA + 300],
                                 start=True, stop=False)
                nc.tensor.matmul(hp[:, s, 0:300], lhsT=h3s[0:100, 1, c0:c1],
                                 rhs=wp_s[0:100, WP_WHB:WP_WHB + 300],
                                 start=False, stop=True)
            live[("hp", i)] = hp

        BQ = 4           # tiles per batched score-chain / exp op

        # NOTE: head columns and gumbel are packed D-MAJOR (col = d*25 + g)
        # so every sampling view collapses to <=3D APs (ScalarTensorTensor
        # ISA limit): [p, (q s d), g] with (q s d) contiguous-nested.

        def st_samp(i):
            hp = live.pop(("hp", i))
            q = i % BQ
            if q == 0:
                # one exp over BQ tiles' gumbel amortizes the Act bubble
                gum_s = live[("gum", i // CH)]
                ex_g = samp.tile([128, BQ, NSUB, GD], F32, tag="ex")
                j = i % CH
                nc.scalar.activation(
                    ex_g.rearrange("p c s e -> p (c s e)"),
                    gum_s[:, j:j + BQ].rearrange("p c s e -> p (c s e)"),
                    func=EXP)
                live[("ex", i // BQ)] = ex_g
                # per-4-tile accumulation tiles for the batched score chain
                scsb_t = samp.tile([128, BQ, NSUB, GD], F32, tag="scsb")
                zb_t = samp.tile([128, BQ, NSUB, GD], BF16, tag="zb")
                live[("scsb", i // BQ)] = scsb_t
                live[("zb", i // BQ)] = zb_t
            ex = live[("ex", i // BQ)][:, q]
            scsb = live[("scsb", i // BQ)]
            zb = live[("zb", i // BQ)]

            # rnd expanded over g on the (otherwise idle) Pool engine (bf16)
            rnd_e = samp.tile([128, NSUB, GD], BF16, tag="rnde")
            nc.gpsimd.tensor_copy(
                out=rnd_e.rearrange("p s (d g) -> p (s d) g", d=D),
                in_=rnd_s[:, i * 16:(i + 1) * 16]
                .unsqueeze(2).broadcast_to([128, NSUB * D, G]))

            # per-tile psum drains:
            # scs = pai * exp(gumbel)  (signed score, drains pai)
            nc.vector.tensor_tensor(out=scsb[:, q], in0=hp[:, :, 200:300],
                                    in1=ex, op=OP.mult)
            # asig = |sig| on Act (drains sig), bf16 for the 2x zz mul
            asig = samp.tile([128, NSUB, GD], BF16, tag="asig")
            nc.scalar.activation(asig, hp[:, :, 100:200],
                                 func=mybir.ActivationFunctionType.Abs)
            zz = samp.tile([128, NSUB, GD], BF16, tag="zz")
            nc.vector.tensor_tensor(out=zz, in0=asig, in1=rnd_e, op=OP.mult)
            # z = zz + mu              (drains mu)
            nc.vector.tensor_tensor(out=zb[:, q], in0=zz, in1=hp[:, :, 0:100],
                                    op=OP.add)

        def st_batch(k, phase):
            # advance group k's batched score chain by one op per step so the
            # big ops interleave with the per-tile psum drains (no DVE burst)
            scsb = live[("scsb", k)]
            zb = live[("zb", k)]
            if phase == 0:
                ascs_t = samp.tile([128, BQ, NSUB, GD], F32, tag="ascs")
                nc.vector.scalar_tensor_tensor(
                    out=ascs_t.rearrange("p c s e -> p (c s e)"),
                    in0=scsb.rearrange("p c s e -> p (c s e)"), scalar=-1.0,
                    in1=scsb.rearrange("p c s e -> p (c s e)"),
                    op0=OP.mult, op1=OP.max)
                live[("ascs", k)] = ascs_t
            elif phase == 1:
                ascs = live[("ascs", k)]
                smax_t = samp.tile([128, BQ * NSUB * D], F32, tag="smax")
                nc.vector.tensor_reduce(
                    smax_t, ascs.rearrange("p c s (d g) -> p (c s d) g", d=D),
                    axis=AX, op=OP.max)
                oh_t = samp.tile([128, BQ, NSUB, GD], BF16, tag="oh")
                nc.vector.tensor_tensor(
                    out=oh_t.rearrange("p c s (d g) -> p (c s d) g", d=D),
                    in0=ascs.rearrange("p c s (d g) -> p (c s d) g", d=D),
                    in1=smax_t.unsqueeze(2)
                    .broadcast_to([128, BQ * NSUB * D, G]),
                    op=OP.is_ge)
                live[("oh", k)] = oh_t
                live.pop(("ascs", k))
            elif phase == 2:
                oh = live[("oh", k)]
                zoh_t = samp.tile([128, BQ, NSUB, GD], BF16, tag="zoh")
                nc.vector.tensor_tensor(
                    out=zoh_t.rearrange("p c s e -> p (c s e)"),
                    in0=zb.rearrange("p c s e -> p (c s e)"),
                    in1=oh.rearrange("p c s e -> p (c s e)"), op=OP.mult)
                live[("zoh", k)] = zoh_t
                live.pop(("oh", k))
            else:
                zoh = live.pop(("zoh", k))
                nc.vector.tensor_reduce(
                    outacc[:, k * BQ * 16:(k + 1) * BQ * 16],
                    zoh.rearrange("p c s (d g) -> p (c s d) g", d=D),
                    axis=AX, op=OP.add)
                live.pop(("scsb", k))
                live.pop(("zb", k))

        # software-pipelined emission: step i runs h1(i), h2(i-1), h3(i-2),
        # heads(i-3), sampling(i-3); gumbel group DMAs prefetch ahead.
        for step in range(NT + 3):
            if step < NT and step % CH == 0:
                st_gum(step // CH)
            if 1 <= step < NT + 1:
                st_h2(step - 1)
            if 2 <= step < NT + 2:
                st_h3(step - 2)
            if step < NT:
                st_h1(step)
            if 3 <= step:
                i = step - 3
                st_heads(i)
                st_samp(i)
                if i >= BQ:
                    st_batch(i // BQ - 1, i % BQ)
        # flush the last group's batch chain
        for ph in range(BQ):
            st_batch(NT // BQ - 1, ph)

        nc.sync.dma_start(out=out_d[:, :], in_=outacc)

    _split_multi_waits(nc)
    return nc


def _pack_weights(W1, b1, W2, b2, W3, b3, Wmu, bmu, Wsig, bsig, Wpai, bpai):
    # WH: [200, 300] stacked head weights, col = head*100 + d*25 + g (D-MAJOR)
    WH = np.zeros((H3, 300), np.float32)
    bh = np.zeros((300,), np.float32)
    for hd, (W, b) in enumerate([(Wmu, bmu), (Wsig, bsig), (Wpai, bpai)]):
        Wdm = np.asarray(W, np.float32).transpose(1, 0, 2)       # [D, G, H3]
        WH[:, hd * GD:(hd + 1) * GD] = Wdm.reshape(GD, H3).T
        bh[hd * GD:(hd + 1) * GD] = np.asarray(b, np.float32).T.reshape(GD)
    # bias hi/lo split: hi exactly representable at 10 mantissa bits (fp32r)
    bh_hi = (bh.view(np.uint32) & np.uint32(0xFFFFE000)).view(np.float32)
    bh_lo = bh - bh_hi

    wpk = np.zeros((128, WP_COLS), np.float32)
    w1t = np.asarray(W1, np.float32).T            # [3, 128]
    for a in range(8):
        wpk[4 * a:4 * a + 3, WP_W1 + 128 * a:WP_W1 + 128 * (a + 1)] = w1t
        wpk[64 + 4 * a:64 + 4 * a + 3, WP_W1 + 128 * a:WP_W1 + 128 * (a + 1)] = w1t
    wpk[0:128, WP_W2:WP_W2 + 256] = np.asarray(W2, np.float32).T
    w3t = np.asarray(W3, np.float32).T            # [256, 200]
    wpk[0:128, WP_W3A:WP_W3A + 200] = w3t[0:128]
    wpk[0:128, WP_W3B:WP_W3B + 200] = w3t[128:256]
    wpk[0:100, WP_WHA:WP_WHA + 300] = WH[0:100]
    wpk[100, WP_WHA:WP_WHA + 300] = bh_hi
    wpk[101, WP_WHA:WP_WHA + 300] = bh_lo
    wpk[0:100, WP_WHB:WP_WHB + 300] = WH[100:200]
    wpk[0, WP_ONES:WP_ONES + 512] = 1.0
    wpk[0, WP_B3:WP_B3 + 200] = np.asarray(b3, np.float32)

    bpk = np.zeros((128, 5), np.float32)
    bpk[:, 0] = np.asarray(b1, np.float32)
    b2 = np.asarray(b2, np.float32)
    bpk[:, 1] = b2[0:128]
    bpk[:, 2] = b2[128:256]
    b3 = np.asarray(b3, np.float32)
    bpk[0:100, 3] = b3[0:100]
    bpk[0:100, 4] = b3[100:200]
    return np.ascontiguousarray(wpk), np.ascontiguousarray(bpk)


_NC_CACHE = None
LAST_RESULT = None


def kernel(x0, rand, gumbel, W1, b1, W2, b2, W3, b3,
           Wmu, bmu, Wsig, bsig, Wpai, bpai):
    global _NC_CACHE, LAST_RESULT
    if _NC_CACHE is None:
        _NC_CACHE = _build_nc()
    nc = _NC_CACHE

    x0 = np.asarray(x0, np.float32)
    rand = np.asarray(rand, np.float32)
    gumbel = np.asarray(gumbel, np.float32)

    wpk, bpk = _pack_weights(W1, b1, W2, b2, W3, b3,
                             Wmu, bmu, Wsig, bsig, Wpai, bpai)

    in_maps = []
    for c in range(NCORES):
        sl = slice(c * BS, (c + 1) * BS)
        xc = x0[sl]                                   # [32768, 3]
        # chunk c (1024 rows): stripe t=c//16 (cols 1024t:+1024), band
        # q=(c%16)//8 (partitions 64q+...), variant a=c%8 (rows 64q+4a+f)
        xq4 = xc.reshape(2, 2, 8, 1024, 3)            # [t, q, a, jj, f]
        xpk = np.zeros((96, 2048), np.float32)
        for t in range(2):
            for q in range(2):
                blk = xq4[t, q].transpose(0, 2, 1)    # [a, f, jj]
                blk = np.concatenate(
                    [blk, np.zeros((8, 1, 1024), np.float32)], axis=1)
                xpk[64 * q:64 * q + 32, 1024 * t:1024 * (t + 1)] = \
                    blk.reshape(32, 1024)
        # d-major gumbel columns (e = d*25 + g), rows r = 512*it+128*s+p
        gc = gumbel[sl].transpose(0, 2, 1).reshape(BS, GD)
        gpk = np.ascontiguousarray(
            gc.reshape(NT, NSUB, 128, GD).transpose(2, 0, 1, 3)
            .reshape(128, NT * NSUB * GD))
        rc = rand[sl]
        rpk = np.ascontiguousarray(
            rc.reshape(NT, NSUB, 128, D).transpose(2, 0, 1, 3)
            .reshape(128, NT * NSUB * D))
        in_maps.append({"xp": xpk, "wp": wpk, "bp": bpk,
                        "gum": gpk, "rnd": rpk,
                        "ones2": np.ones((2, 512), np.float32)})

    res = run_bass_kernel_spmd(nc, in_maps, core_ids=list(range(NCORES)))
    LAST_RESULT = res
    outs = []
    for c in range(NCORES):
        o = res.results[c]["out"]                     # [128, 1024]
        outs.append(o.reshape(128, NT, NSUB, D).transpose(1, 2, 0, 3)
                    .reshape(BS, D))
    return np.ascontiguousarray(np.concatenate(outs, axis=0).astype(np.float32))


# revision 36
# speedup vs baseline: 1.1251x; 1.1251x over previous
"""Trainium2 Bass kernel for nn_Backward_12094627905824 (MLP trunk + gumbel-argmax
mixture sampling). Data-parallel over 8 NeuronCores: batch B=262144 is sharded
32768 rows/core; the small MLP / head weights are replicated.

Math per batch row b (reference semantics):
  h = relu chain: 3 -> 128 -> 256 -> 200
  mu/sig/pai[g,d] = heads (25 comps x 4 dims), pai/sigma through abs
  idx[d] = argmax_g log(pai+1e-12) + gumbel[b,g,d]
  out[b,d] = rand[b,d]*|sig[idx,d]| + mu[idx,d]

On-device reformulation (argmax-invariant): score = |pai_raw| * exp(gumbel);
selection via one-hot (score >= rowmax); out = sum_g onehot * z where
z = mu + rand*|sig| is folded BEFORE the masked reduce (one select instead of
two).

v2 engine plan (per 512-row tile, 64 tiles/core):
  PE   : 13 matmuls, all biases folded in (x-pack carries no ones; trunk
         biases are added exactly in f32 by Pool/Act; head biases enter via
         two ones-rows in the h3a SBUF tile against bias hi/lo rows of the
         packed head weights - same precision as the baseline's hi/lo ones
         matmul, but zero extra PE cycles).
  Act  : h2a/h2b fused bias+relu (psum->sbuf), exp(gumbel).
  Pool : h1/h3a/h3b fused bias+relu via tensor_scalar(add bias, max 0),
         |pai|, |sig| via abs_max, z1 = |sig|*rand, z = z1 + mu.
  DVE  : score mul, rowmax reduce, one-hot is_ge, z*onehot, masked-sum.
  DMA  : everything batched: x/weights/biases/rand/out are ONE DMA each
         (host-side relayout packs them 128-partitions-tall), gumbel in 8
         contiguous group DMAs of 8 tiles each.
"""
import numpy as np

import concourse.bass as bass
import concourse.mybir as mybir
import bass_rust
from concourse.tile import TileContext
from concourse.bass_utils import run_bass_kernel_spmd

NCORES = 8
B, G, D = 262144, 25, 4
GD = G * D                       # 100
H1, H2, H3 = 128, 256, 200
BS = B // NCORES                 # 32768 rows per core
NB = 512                         # batch columns per compute tile
NT = BS // NB                    # 64 tiles
NSUB = NB // 128                 # 4 sub-blocks of 128 rows
CH = 8                           # tiles per gumbel DMA group
NGRP = NT // CH                  # 8 groups

F32 = mybir.dt.float32
F32R = mybir.dt.float32r
BF16 = mybir.dt.bfloat16

# weight-pack column map (f32r matmul operands, 128 partitions)
# W1 region: 8 shifted variants (K=32 trick): variant a has W1.T in rows
# 4a..4a+2 of a [32,128] block, zeros elsewhere; replicated at bases 0 and 64
# so lhsT.base matches the x rhs base (PE quadrant rule).
WP_W1 = 0          # [{0:32,64:96}, 0:1024]  8 x [32,128] shifted W1.T blocks
WP_W2 = 1024       # [0:128, +0:256]  W2.T (h2a cols 0:128, h2b cols 128:256)
WP_W3A = 1280      # [0:128, +0:200]  W3.T[h2 feats 0:128]  (h3a 0:100, h3b 100:200)
WP_W3B = 1480      # [0:128, +0:200]  W3.T[h2 feats 128:256]
WP_WHA = 1680      # [0:102, +0:300]  heads for h3 feats 0:100 + bias hi/lo rows
WP_WHB = 1980      # [0:100, +0:300]  heads for h3 feats 100:200
WP_COLS = 2280


def _split_multi_waits(nc):
    # walrus CoreV3 codegen accepts only one sync-wait per instruction; Tile's
    # exit drain waits once per active proc. Split into single-wait drains.
    for bb in nc.main_func.blocks:
        insts = list(bb.instructions)
        out = []
        changed = False
        for inst in insts:
            si = inst.sync_info
            if si is not None and len(si.on_wait) > 1:
                waits = list(si.on_wait)
                for k, w in enumerate(waits[:-1]):
                    d = mybir.InstDrain(name=f"{inst.name}-sw{k}", ins=[], outs=[])
                    d.engine = inst.engine
                    d.sync_info = bass_rust.SyncInfo(on_wait=[w], on_update=[])
                    nc.register_instruction(d)
                    out.append(d)
                si.on_wait = [waits[-1]]
                changed = True
            out.append(inst)
        if changed:
            bb.instructions = out


def _build_nc():
    nc = bass.Bass(trn_type="TRN2")

    xp = nc.dram_tensor("xp", [96, 2048], F32R, kind="ExternalInput")
    ones2 = nc.dram_tensor("ones2", [2, 512], F32R, kind="ExternalInput")
    wp = nc.dram_tensor("wp", [128, WP_COLS], F32R, kind="ExternalInput")
    bp = nc.dram_tensor("bp", [128, 5], F32, kind="ExternalInput")
    gum = nc.dram_tensor("gum", [128, NT * NSUB * GD], F32, kind="ExternalInput")
    rnd = nc.dram_tensor("rnd", [128, NT * NSUB * D], F32, kind="ExternalInput")
    out_d = nc.dram_tensor("out", [128, NT * NSUB * D], F32, kind="ExternalOutput")

    AX = mybir.AxisListType.X
    OP = mybir.AluOpType
    RELU = mybir.ActivationFunctionType.Relu
    EXP = mybir.ActivationFunctionType.Exp

    from contextlib import ExitStack
    with TileContext(nc) as tc, ExitStack() as ctx:
        const = ctx.enter_context(tc.tile_pool(name="const", bufs=1))
        io = ctx.enter_context(tc.tile_pool(name="io", bufs=2))
        act = ctx.enter_context(tc.tile_pool(name="act", bufs=3))
        samp = ctx.enter_context(tc.tile_pool(name="samp", bufs=3))
        # trunk psum: 4 rotating 1-bank tiles hold h1p/h2ap/h2bp/h3ap/h3bp
        pT = ctx.enter_context(tc.tile_pool(name="pT", bufs=4, space="PSUM"))
        # heads psum: one 4-bank tile, freed by the per-tile drain ops
        pHD = ctx.enter_context(tc.tile_pool(name="pHD", bufs=1, space="PSUM"))

        # --- load packed inputs (Act queue: x/weights/biases/rand; SP: gum) ---
        xp_s = const.tile([96, 2048], F32R, tag="xp")
        nc.scalar.dma_start(out=xp_s, in_=xp[:, :])
        wp_s = const.tile([128, WP_COLS], F32R, tag="wp")
        nc.scalar.dma_start(out=wp_s, in_=wp[:, :])
        bp_s = const.tile([128, 5], F32, tag="bp")
        nc.scalar.dma_start(out=bp_s, in_=bp[:, :])
        rnd_s = const.tile([128, NT * NSUB * D], F32, tag="rnd")
        nc.scalar.dma_start(out=rnd_s, in_=rnd[:, :])
        outacc = const.tile([128, NT * NSUB * D], F32, tag="outacc")

        # rotating-state dicts keyed by tile index
        live = {}

        def st_gum(g):
            gum_s = io.tile([128, CH, NSUB, GD], F32, tag="gum")
            nc.sync.dma_start(
                out=gum_s,
                in_=gum[:, g * CH * NSUB * GD:(g + 1) * CH * NSUB * GD]
                .rearrange("p (c s e) -> p c s e", c=CH, s=NSUB))
            live[("gum", g)] = gum_s

        def st_h1(i):
            c2 = i // 2
            xt, xq, xa = c2 // 16, (c2 % 16) // 8, c2 % 8
            xcol = 1024 * xt + (i % 2) * 512
            h1p = pT.tile([128, 512], F32, tag="pt")
            nc.tensor.matmul(h1p,
                             lhsT=wp_s[64 * xq:64 * xq + 32,
                                       WP_W1 + 128 * xa:WP_W1 + 128 * (xa + 1)],
                             rhs=xp_s[64 * xq:64 * xq + 32, xcol:xcol + 512],
                             start=True, stop=True)
            h1s = act.tile([128, 512], F32R, tag="h1")
            nc.scalar.activation(h1s, h1p, func=RELU, bias=bp_s[:, 0:1], scale=1.0)
            live[("h1", i)] = h1s

        def st_h2(i):
            h1s = live.pop(("h1", i))
            h2ap = pT.tile([128, 512], F32, tag="pt")
            nc.tensor.matmul(h2ap, lhsT=wp_s[0:128, WP_W2:WP_W2 + 128],
                             rhs=h1s, start=True, stop=True)
            h2bp = pT.tile([128, 512], F32, tag="pt")
            nc.tensor.matmul(h2bp, lhsT=wp_s[0:128, WP_W2 + 128:WP_W2 + 256],
                             rhs=h1s, start=True, stop=True)
            h2s = act.tile([128, 2, 512], F32R, tag="h2")
            nc.scalar.activation(h2s[:, 0, :], h2ap, func=RELU,
                                 bias=bp_s[:, 1:2], scale=1.0)
            nc.scalar.activation(h2s[:, 1, :], h2bp, func=RELU,
                                 bias=bp_s[:, 2:3], scale=1.0)
            live[("h2", i)] = h2s

        def st_h3(i):
            h2s = live.pop(("h2", i))
            h3ap = pT.tile([128, 512], F32, tag="pt")
            nc.tensor.matmul(h3ap[0:100, :], lhsT=wp_s[0:128, WP_W3A:WP_W3A + 100],
                             rhs=h2s[:, 0, :], start=True, stop=False)
            nc.tensor.matmul(h3ap[0:100, :], lhsT=wp_s[0:128, WP_W3B:WP_W3B + 100],
                             rhs=h2s[:, 1, :], start=False, stop=True)
            h3bp = pT.tile([128, 512], F32, tag="pt")
            nc.tensor.matmul(h3bp[0:100, :], lhsT=wp_s[0:128, WP_W3A + 100:WP_W3A + 200],
                             rhs=h2s[:, 0, :], start=True, stop=False)
            nc.tensor.matmul(h3bp[0:100, :], lhsT=wp_s[0:128, WP_W3B + 100:WP_W3B + 200],
                             rhs=h2s[:, 1, :], start=False, stop=True)
            h3sa = act.tile([102, 512], F32R, tag="h3a")
            if i < 3:
                # ones rows for the head-bias hi/lo trick; each of the 3
                # rotating buffers is initialized once and never clobbered
                # (the relu below only writes rows 0:100). DMA because
                # vector-engine writes must start at partition 0/32/64/96.
                nc.scalar.dma_start(out=h3sa[100:102, :], in_=ones2[:, :])
            nc.scalar.activation(h3sa[0:100, :], h3ap[0:100, :], func=RELU,
                                 bias=bp_s[0:100, 3:4], scale=1.0)
            h3sb = act.tile([100, 512], F32R, tag="h3b")
            nc.scalar.activation(h3sb, h3bp[0:100, :], func=RELU,
                                 bias=bp_s[0:100, 4:5], scale=1.0)
            live[("h3", i)] = (h3sa, h3sb)

        def st_heads(i):
            h3sa, h3sb = live.pop(("h3", i))
            hp = pHD.tile([128, NSUB, 512], F32, tag="hp")
            for s in range(NSUB):
                c0, c1 = s * 128, (s + 1) * 128
                nc.tensor.matmul(hp[:, s, 0:300], lhsT=h3sa[:, c0:c1],
                                 rhs=wp_s[0:102, WP_WHA:WP_WHA + 300],
                                 start=True, stop=False)
                nc.tensor.matmul(hp[:, s, 0:300], lhsT=h3sb[:, c0:c1],
                                 rhs=wp_s[0:100, WP_WHB:WP_WHB + 300],
                                 start=False, stop=True)
            live[("hp", i)] = hp

        EXG = 4          # tiles per batched exp op

        # NOTE: head columns and gumbel are packed D-MAJOR (col = d*25 + g)
        # so every sampling view collapses to <=3D APs (ScalarTensorTensor
        # ISA limit): [p, (s d), g] with (s d) contiguous-nested.

        def st_samp(i):
            hp = live.pop(("hp", i))
            if i % EXG == 0:
                # one exp over EXG tiles' gumbel amortizes the Act bubble
                gum_s = live[("gum", i // CH)]
                ex_g = samp.tile([128, EXG, NSUB, GD], F32, tag="ex")
                j = i % CH
                nc.scalar.activation(
                    ex_g.rearrange("p c s e -> p (c s e)"),
                    gum_s[:, j:j + EXG].rearrange("p c s e -> p (c s e)"),
                    func=EXP)
                live[("ex", i // EXG)] = ex_g
            ex = live[("ex", i // EXG)][:, i % EXG]

            # rnd expanded over g on the (otherwise idle) Pool engine (bf16)
            rnd_e = samp.tile([128, NSUB, GD], BF16, tag="rnde")
            nc.gpsimd.tensor_copy(
                out=rnd_e.rearrange("p s (d g) -> p (s d) g", d=D),
                in_=rnd_s[:, i * 16:(i + 1) * 16]
                .unsqueeze(2).broadcast_to([128, NSUB * D, G]))

            # scs = pai * exp(gumbel)   (signed score; also drains pai psum)
            scs = samp.tile([128, NSUB, GD], F32, tag="scs")
            nc.vector.tensor_tensor(out=scs, in0=hp[:, :, 200:300], in1=ex,
                                    op=OP.mult)
            # ascs = |scs| via the stt (x*-1) max x idiom (all SBUF)
            ascs = samp.tile([128, NSUB, GD], F32, tag="ascs")
            nc.vector.scalar_tensor_tensor(out=ascs, in0=scs, scalar=-1.0,
                                           in1=scs, op0=OP.mult, op1=OP.max)
            # asig = |sig| on Act (drains sig psum), bf16 for the 2x zz mul
            asig = samp.tile([128, NSUB, GD], BF16, tag="asig")
            nc.scalar.activation(asig, hp[:, :, 100:200],
                                 func=mybir.ActivationFunctionType.Abs)
            zz = samp.tile([128, NSUB, GD], BF16, tag="zz")
            nc.vector.tensor_tensor(out=zz, in0=asig, in1=rnd_e, op=OP.mult)
            # z = zz + mu               (drains mu psum)
            z = samp.tile([128, NSUB, GD], BF16, tag="z")
            nc.vector.tensor_tensor(out=z, in0=zz, in1=hp[:, :, 0:100],
                                    op=OP.add)

            # smax = max_g |scs|
            smax = samp.tile([128, NSUB * D], F32, tag="smax")
            nc.vector.tensor_reduce(
                smax, ascs.rearrange("p s (d g) -> p (s d) g", d=D),
                axis=AX, op=OP.max)
            # oh = (|scs| >= smax)
            oh = samp.tile([128, NSUB, GD], BF16, tag="oh")
            nc.vector.tensor_tensor(
                out=oh.rearrange("p s (d g) -> p (s d) g", d=D),
                in0=ascs.rearrange("p s (d g) -> p (s d) g", d=D),
                in1=smax.unsqueeze(2).broadcast_to([128, NSUB * D, G]),
                op=OP.is_ge)
            # zoh = z * oh (bf16 2x), sel = sum_g zoh -> outacc (f32)
            zoh = samp.tile([128, NSUB, GD], BF16, tag="zoh")
            nc.vector.tensor_tensor(out=zoh, in0=z, in1=oh, op=OP.mult)
            nc.vector.tensor_reduce(
                outacc[:, i * 16:(i + 1) * 16],
                zoh.rearrange("p s (d g) -> p (s d) g", d=D),
                axis=AX, op=OP.add)

        # software-pipelined emission: step i runs h1(i), h2(i-1), h3(i-2),
        # heads(i-3), sampling(i-3); gumbel group DMAs prefetch ahead.
        for step in range(NT + 3):
            if step < NT and step % CH == 0:
                st_gum(step // CH)
            if 1 <= step < NT + 1:
                st_h2(step - 1)
            if 2 <= step < NT + 2:
                st_h3(step - 2)
            if step < NT:
                st_h1(step)
            if 3 <= step:
                st_heads(step - 3)
                st_samp(step - 3)

        nc.sync.dma_start(out=out_d[:, :], in_=outacc)

    _split_multi_waits(nc)
    return nc


def _pack_weights(W1, b1, W2, b2, W3, b3, Wmu, bmu, Wsig, bsig, Wpai, bpai):
    # WH: [200, 300] stacked head weights, col = head*100 + d*25 + g (D-MAJOR)
    WH = np.zeros((H3, 300), np.float32)
    bh = np.zeros((300,), np.float32)
    for hd, (W, b) in enumerate([(Wmu, bmu), (Wsig, bsig), (Wpai, bpai)]):
        Wdm = np.asarray(W, np.float32).transpose(1, 0, 2)       # [D, G, H3]
        WH[:, hd * GD:(hd + 1) * GD] = Wdm.reshape(GD, H3).T
        bh[hd * GD:(hd + 1) * GD] = np.asarray(b, np.float32).T.reshape(GD)
    # bias hi/lo split: hi exactly representable at 10 mantissa bits (fp32r)
    bh_hi = (bh.view(np.uint32) & np.uint32(0xFFFFE000)).view(np.float32)
    bh_lo = bh - bh_hi

    wpk = np.zeros((128, WP_COLS), np.float32)
    w1t = np.asarray(W1, np.float32).T            # [3, 128]
    for a in range(8):
        wpk[4 * a:4 * a + 3, WP_W1 + 128 * a:WP_W1 + 128 * (a + 1)] = w1t
        wpk[64 + 4 * a:64 + 4 * a + 3, WP_W1 + 128 * a:WP_W1 + 128 * (a + 1)] = w1t
    wpk[0:128, WP_W2:WP_W2 + 256] = np.asarray(W2, np.float32).T
    w3t = np.asarray(W3, np.float32).T            # [256, 200]
    wpk[0:128, WP_W3A:WP_W3A + 200] = w3t[0:128]
    wpk[0:128, WP_W3B:WP_W3B + 200] = w3t[128:256]
    wpk[0:100, WP_WHA:WP_WHA + 300] = WH[0:100]
    wpk[100, WP_WHA:WP_WHA + 300] = bh_hi
    wpk[101, WP_WHA:WP_WHA + 300] = bh_lo
    wpk[0:100, WP_WHB:WP_WHB + 300] = WH[100:200]

    bpk = np.zeros((128, 5), np.float32)
    bpk[:, 0] = np.asarray(b1, np.float32)
    b2 = np.asarray(b2, np.float32)
    bpk[:, 1] = b2[0:128]
    bpk[:, 2] = b2[128:256]
    b3 = np.asarray(b3, np.float32)
    bpk[0:100, 3] = b3[0:100]
    bpk[0:100, 4] = b3[100:200]
    return np.ascontiguousarray(wpk), np.ascontiguousarray(bpk)


_NC_CACHE = None
LAST_RESULT = None


def kernel(x0, rand, gumbel, W1, b1, W2, b2, W3, b3,
           Wmu, bmu, Wsig, bsig, Wpai, bpai):
    global _NC_CACHE, LAST_RESULT
    if _NC_CACHE is None:
        _NC_CACHE = _build_nc()
    nc = _NC_CACHE

    x0 = np.asarray(x0, np.float32)
    rand = np.asarray(rand, np.float32)
    gumbel = np.asarray(gumbel, np.float32)

    wpk, bpk = _pack_weights(W1, b1, W2, b2, W3, b3,
                             Wmu, bmu, Wsig, bsig, Wpai, bpai)

    in_maps = []
    for c in range(NCORES):
        sl = slice(c * BS, (c + 1) * BS)
        xc = x0[sl]                                   # [32768, 3]
        # chunk c (1024 rows): stripe t=c//16 (cols 1024t:+1024), band
        # q=(c%16)//8 (partitions 64q+...), variant a=c%8 (rows 64q+4a+f)
        xq4 = xc.reshape(2, 2, 8, 1024, 3)            # [t, q, a, jj, f]
        xpk = np.zeros((96, 2048), np.float32)
        for t in range(2):
            for q in range(2):
                blk = xq4[t, q].transpose(0, 2, 1)    # [a, f, jj]
                blk = np.concatenate(
                    [blk, np.zeros((8, 1, 1024), np.float32)], axis=1)
                xpk[64 * q:64 * q + 32, 1024 * t:1024 * (t + 1)] = \
                    blk.reshape(32, 1024)
        # d-major gumbel columns (e = d*25 + g), rows r = 512*it+128*s+p
        gc = gumbel[sl].transpose(0, 2, 1).reshape(BS, GD)
        gpk = np.ascontiguousarray(
            gc.reshape(NT, NSUB, 128, GD).transpose(2, 0, 1, 3)
            .reshape(128, NT * NSUB * GD))
        rc = rand[sl]
        rpk = np.ascontiguousarray(
            rc.reshape(NT, NSUB, 128, D).transpose(2, 0, 1, 3)
            .reshape(128, NT * NSUB * D))
        in_maps.append({"xp": xpk, "wp": wpk, "bp": bpk,
                        "gum": gpk, "rnd": rpk,
                        "ones2": np.ones((2, 512), np.float32)})

    res = run_bass_kernel_spmd(nc, in_maps, core_ids=list(range(NCORES)))
    LAST_RESULT = res
    outs = []
    for c in range(NCORES):
        o = res.results[c]["out"]                     # [128, 1024]
        outs.append(o.reshape(128, NT, NSUB, D).transpose(1, 2, 0, 3)
                    .reshape(BS, D))
    return np.ascontiguousarray(np.concatenate(outs, axis=0).astype(np.float32))
